# revision 1
# baseline (speedup 1.0000x reference)
"""Trainium2 Bass kernel for nn_BiDirectionalFusionModule.

Computation (B=4, C=256, CK=32, H=W=64, N=4096):
  two DANet-style non-local attentions (d2r: Q from rgb, K/V on depth;
  r2d: swapped), residual with gamma scaling, channel concat, 3x3 conv
  (512->256) + BN(eval) + ReLU.

Sharding: 8 cores = (batch b, image half). Each core computes BOTH attention
directions for its 34-row query slab (32 output rows + 1 halo row each side,
out-of-range rows zero padded) and then the 3x3 conv for its 32 output rows.
No cross-core communication; the host gathers (B,256,64,64) at the end.

Device layout: scores are computed transposed, S^T = k^T q with j (key
index) on partitions and i (query index) free, so E^T=exp(S^T) is directly
the moving operand of the apply matmul, and v^T comes from a DoubleRow fp8
projection with the channel dim interleaved [p, t] (t = channel half).
Zero on-device transposes.

v2 changes vs the 344us baseline:
  - f_kv is fp8 on host, channel-interleaved (128, 2, N): the K and V
    projections become single DoubleRow matmuls (K=256 contraction in one
    pass), halving their PE time and the f_kv DMA bytes.
  - softmax column sums use a DoubleRow fp8 ones-matmul with M=1 that
    accumulates all 16 j-chunk pairs straight into a (1, iw) PSUM strip --
    2 matmuls per pack instead of 4, and no fold matmul / bf16 copy.
  - f_q DMA is sliced per i-block and interleaved with the f_kv blocks so
    the PE starts earlier.

All other matmuls bf16 (scores row-packed 4x via tile_position) or fp8
DoubleRow (apply). Host pre-folds: gamma into wv, BN into conv weights/bias,
4x replication into wq/wk.
"""

import numpy as np
import ml_dtypes

BF16 = ml_dtypes.bfloat16
F8 = ml_dtypes.float8_e4m3fn

B, C, H, W = 4, 256, 64, 64
N = H * W            # 4096 tokens
CK = 32
NI = 34 * 64         # 2176 query positions per core (34 rows incl. halo)
WP = 68              # padded row width: 2 zero cols each side
NPAD = 34 * WP       # 2312
NOUT = 32 * 64       # 2048 output positions per core
NJC = N // 128       # 32 j-chunks
IBLKS = [(0, 512), (512, 512), (1024, 512), (1536, 512), (2048, 128)]

_CACHE = {}
LAST_RESULTS = None


def _build_program():
    import concourse.tile as tile
    from concourse import bacc, mybir

    f32 = mybir.dt.float32
    bf = mybir.dt.bfloat16
    f8 = mybir.dt.float8e4
    Alu = mybir.AluOpType
    Act = mybir.ActivationFunctionType
    DR = mybir.MatmulPerfMode.DoubleRow
    # exp(S - EXP_SHIFT): keeps E=exp(S') inside fp8e4m3 range; softmax
    # normalization cancels the constant exactly.
    EXP_SHIFT = -2.0

    nc = bacc.Bacc("TRN2", debug=False, enable_asserts=False, num_devices=8)

    # ---- DRAM I/O (per-core data, same names on every core) ----
    d_fq34 = [nc.dram_tensor(f"fq34_{d}", (C, NI), bf, kind="ExternalInput").ap()
              for d in range(2)]
    # f_kv fp8, channel-interleaved: (p, t, n) = channel t*128+p, token n
    d_fkv = [nc.dram_tensor(f"fkv_{d}", (128, 2, N), f8, kind="ExternalInput").ap()
             for d in range(2)]
    # f_kv fp8, token-major pair-interleaved for the G-apply stationary:
    # (p, pair, t, c) = channel c, token 256*pair + 128*t + p
    d_fkvT = [nc.dram_tensor(f"fkvT_{d}", (128, 16, 2, 256), f8,
                             kind="ExternalInput").ap()
              for d in range(2)]
    # bf16 q-projection weights: [q4T cc0 | q4T cc1]
    d_attwq = [nc.dram_tensor(f"attwq_{d}", (128, 256), bf, kind="ExternalInput").ap()
               for d in range(2)]
    # fp8 interleaved k/v weights: [k4T_il (2,128) | wvT_il (2,256)]
    d_attwkv = [nc.dram_tensor(f"attwkv_{d}", (128, 768), f8, kind="ExternalInput").ap()
                for d in range(2)]
    d_scal = nc.dram_tensor("scal", (128, 8), f32, kind="ExternalInput").ap()
    d_convw = nc.dram_tensor("convw", (128, 72 * 128), bf, kind="ExternalInput").ap()
    d_convb = nc.dram_tensor("convb", (128, 2), f32, kind="ExternalInput").ap()
    d_mask = nc.dram_tensor("mask", (1, NI), f32, kind="ExternalInput").ap()
    d_y = nc.dram_tensor("y", (C, NOUT), bf, kind="ExternalOutput").ap()

    with tile.TileContext(nc) as tc:
        with (
            tc.tile_pool(name="consts", bufs=1) as consts,
            tc.tile_pool(name="big", bufs=1) as big,
            tc.tile_pool(name="stream", bufs=6) as stream,
            tc.tile_pool(name="kq", bufs=2) as kqp,
            tc.tile_pool(name="vt", bufs=1) as vtp,
            tc.tile_pool(name="Ep", bufs=16) as Ep,
            tc.tile_pool(name="small", bufs=3) as small,
            tc.tile_pool(name="yp", bufs=1) as yp,
        ):
            # ---- constants / inputs resident in SBUF ----
            # DMA priority: the very first PE work is dir-0's k-projection
            # (attwkv_0 + fkv block 0) — queue those bytes first.
            attwq, attwkv = [], []
            for d in range(2):
                attwq.append(consts.tile([128, 256], bf, name=f"attwq{d}",
                                         tag=f"attwq{d}"))
                attwkv.append(consts.tile([128, 768], f8, name=f"attwkv{d}",
                                          tag=f"attwkv{d}"))
            nc.sync.dma_start(attwkv[0][:], d_attwkv[0])
            nc.sync.dma_start(attwq[0][:], d_attwq[0])
            scal = consts.tile([128, 8], f32, name="scal_sb", tag="scal_sb")
            nc.sync.dma_start(scal[:], d_scal)
            mask = consts.tile([1, NI], f32, name="mask_sb", tag="mask_sb")
            # conv weights are not needed until the very end — DMA them late
            # (emitted after dir-0 projections) so they don't delay the start.
            convw = consts.tile([128, 72 * 128], bf, name="convw_sb", tag="convw_sb")
            convb = consts.tile([128, 2], f32, name="convb_sb", tag="convb_sb")

            # DoubleRow ones for the column-sum matmul: [128, 2, 16] view,
            # only [:, :, 0:1] used (16-col pad keeps the t-step 16B aligned).
            ones_dr = consts.tile([128, 32], f8, name="ones_dr", tag="ones_dr")
            nc.vector.memset(ones_dr[:], 1.0)
            ones_dr3 = ones_dr.rearrange("p (t x) -> p t x", t=2)
            onesk1 = consts.tile([1, 128], bf, name="onesk1", tag="onesk1")
            nc.vector.memset(onesk1[:], 1.0)
            expbias = consts.tile([128, 1], f32, name="expbias", tag="expbias")
            nc.vector.memset(expbias[:], EXP_SHIFT)

            fq34 = [[big.tile([128, NI], bf, name=f"fq34_{d}_{cc}",
                              tag=f"fq34_{d}_{cc}")
                     for cc in range(2)] for d in range(2)]

            enh = []
            for d in range(2):
                row = []
                for cc in range(2):
                    t = big.tile([128, NPAD], bf, name=f"enh_{d}_{cc}",
                                 tag=f"enh_{d}_{cc}")
                    nc.gpsimd.memset(t[:], 0.0)
                    row.append(t)
                enh.append(row)

            y_sb = [yp.tile([128, NOUT], bf, name=f"y{oc}", tag=f"y{oc}")
                    for oc in range(2)]

            def attw_q4(d, cc):
                return attwq[d][:, cc * 128:(cc + 1) * 128]

            def attw_k4il(d):   # [128, 2, 128]
                return attwkv[d][:, 0:256].rearrange("p (t m) -> p t m", t=2)

            def attw_vTil(d):   # [128, 2, 256]
                return attwkv[d][:, 256:768].rearrange("p (t c) -> p t c", t=2)

            with tc.tile_pool(name="psA", bufs=1, space="PSUM") as psA:
                k4s, q4s, fkvTs = [], [], []
                # ====== projections for BOTH dirs up front (overlaps the
                # ACT-bound attention of dir 0 with dir 1's projections) ======
                for d in range(2):
                    if d == 1:
                        # dir-1 weights + mask aren't needed until dir-0's
                        # projections are underway — keep them off the
                        # startup-critical DMA path
                        nc.sync.dma_start(attwkv[1][:], d_attwkv[1])
                        nc.sync.dma_start(attwq[1][:], d_attwq[1])
                        nc.sync.dma_start(mask[:], d_mask)
                    k4 = kqp.tile([128, N], bf, name=f"k4_{d}", tag="k4")
                    q4 = kqp.tile([128, NI], bf, name=f"q4_{d}", tag="q4")
                    k4s.append(k4)
                    q4s.append(q4)
                    # token-major f_kv: the G-apply stationary, straight from
                    # DRAM (no V projection on device at all)
                    fkvT = vtp.tile([128, 16 * 512], f8, name=f"fkvT_{d}",
                                    tag=f"fkvT_{d}")
                    fkvTs.append(fkvT.rearrange("p (pr t c) -> p pr t c",
                                                pr=16, t=2))
                    dT = d_fkvT[d].rearrange("p pr t c -> p (pr t c)")
                    for n in range(8):  # 512-wide token blocks of f_kv
                        s3 = stream.tile([128, 2, 512], f8, name=f"s_{d}_{n}",
                                         tag="stream")
                        nc.sync.dma_start(s3[:], d_fkv[d][:, :, n * 512:(n + 1) * 512])
                        # stagger the fq34 DMAs (sliced per i-block) between
                        # the fkv blocks so scores can start early
                        if n < len(IBLKS):
                            i0, iw = IBLKS[n]
                            for cc in range(2):
                                nc.sync.dma_start(
                                    fq34[d][cc][:, i0:i0 + iw],
                                    d_fq34[d][cc * 128:(cc + 1) * 128, i0:i0 + iw])

                        # k4 chunk: one DoubleRow matmul contracts all 256
                        # input channels: [wk;wk;wk;wk] @ f_kv block
                        kp = psA.tile([128, 512], f32, name=f"kp_{d}_{n}",
                                      tag="conv", bufs=1)
                        nc.tensor.matmul(kp[:], attw_k4il(d), s3[:],
                                         perf_mode=DR, start=True, stop=True)
                        nc.vector.tensor_scalar(
                            k4[:, n * 512:(n + 1) * 512], kp[:],
                            scal[:, 4 * d + 1:4 * d + 2], None, Alu.add)

                        # q4 i-block: (128, iw) = [wq;wq;wq;wq] @ f_q34 slice
                        # (bf16: f_q stays bf16 for the residual path)
                        if n < len(IBLKS):
                            i0, iw = IBLKS[n]
                            qp = psA.tile([128, 512], f32, name=f"qp_{d}_{n}",
                                          tag="conv", bufs=1)
                            nc.tensor.matmul(qp[:, :iw], attw_q4(d, 0),
                                             fq34[d][0][:, i0:i0 + iw],
                                             start=True, stop=False)
                            nc.tensor.matmul(qp[:, :iw], attw_q4(d, 1),
                                             fq34[d][1][:, i0:i0 + iw],
                                             start=False, stop=True)
                            nc.vector.tensor_scalar(
                                q4[:, i0:i0 + iw], qp[:, :iw],
                                scal[:, 4 * d:4 * d + 1], None, Alu.add)
                        # fkvT is first needed by the apply (a few us in) —
                        # don't let it delay the k/q projections' DMAs
                        if 4 <= n < 8:
                            nc.sync.dma_start(
                                fkvT[:, (n - 4) * 2048:(n - 3) * 2048],
                                dT[:, (n - 4) * 2048:(n - 3) * 2048])

                nc.sync.dma_start(convw[:], d_convw)
                nc.sync.dma_start(convb[:], d_convb)

                # ================= attention i-blocks =================
                # ib-major, direction-minor: both dirs' enh rows complete
                # block by block, so the conv waves become ready early and
                # can fill the PE slack in the ACT-paced attention phase and
                # the per-block normalization stalls.
                for ib, (i0, iw) in enumerate(IBLKS):
                    for d in range(2):
                        k4, q4, fkvT3 = k4s[d], q4s[d], fkvTs[d]
                        cs1 = psA.tile([1, 512], f32, name=f"cs_{d}_{ib}",
                                       tag="cs", bufs=1)
                        ap_ps = [psA.tile([128, 512], f32, name=f"ap_{d}_{ib}_{cc}",
                                          tag="apply", bufs=2) for cc in range(2)]
                        # fp8 DoubleRow G-apply: one matmul contracts a
                        # jc-PAIR (K=256) of raw f_kv tokens against E —
                        # G[cin, i] = sum_j f_kv[cin, j] E[j, i]; the wv
                        # projection is applied AFTER softmax normalization
                        # (associativity), so there is no V projection
                        # anywhere. The column sum rides the same E3 operand
                        # with a DR ones-matmul into a (1, iw) PSUM strip.
                        def apply_pair(pair, E3):
                            # csum first: its pair-15 stop gates the whole
                            # normalize chain at the block boundary
                            nc.tensor.matmul(
                                cs1[:1, :iw], ones_dr3[:, :, 0:1],
                                E3[:, :, :iw], perf_mode=DR,
                                start=(pair == 0), stop=(pair == 15),
                                skip_group_check=True)
                            for cc in range(2):
                                nc.tensor.matmul(
                                    ap_ps[cc][:, :iw],
                                    fkvT3[:, pair, :, cc * 128:(cc + 1) * 128],
                                    E3[:, :, :iw],
                                    perf_mode=DR,
                                    start=(pair == 0), stop=(pair == 15),
                                    skip_group_check=True)

                        if iw == 512:
                            for p in range(8):
                                # two 2-bank half-packs (bufs=2) so the next
                                # pack's scores can start while this pack's
                                # exp runs — keeps the PE gap-free.
                                halves = [
                                    psA.tile([128, 1024], f32,
                                             name=f"pk_{d}_{ib}_{p}_{h}",
                                             tag="pack", bufs=2)
                                    for h in range(2)
                                ]
                                E = Ep.tile([128, 2048], f8,
                                            name=f"E_{d}_{ib}_{p}", tag="E")
                                for g in range(4):
                                    jc = 4 * p + g
                                    h, hg = divmod(g, 2)
                                    nc.tensor.matmul(
                                        halves[h][:, hg * 512: hg * 512 + iw],
                                        k4[32 * g:32 * g + 32,
                                           jc * 128:(jc + 1) * 128],
                                        q4[32 * g:32 * g + 32, i0:i0 + iw],
                                        start=True, stop=True,
                                        tile_position=(32 * g, 0))
                                for h in range(2):
                                    nc.scalar.activation(
                                        E[:, h * 1024:(h + 1) * 1024],
                                        halves[h][:], Act.Exp,
                                        bias=expbias[:, 0:1])
                                for gp in range(2):
                                    E3 = E[:, gp * 1024:(gp + 1) * 1024].rearrange(
                                        "p (t i) -> p t i", t=2)
                                    apply_pair(2 * p + gp, E3)
                        else:
                            for p in range(8):
                                halves = [
                                    psA.tile([128, 1024], f32,
                                             name=f"pk_{d}_{ib}_{p}_{h}",
                                             tag="pack", bufs=2)
                                    for h in range(2)
                                ]
                                E = Ep.tile([128, 2048], f8,
                                            name=f"E_{d}_{ib}_{p}", tag="E")
                                for g in range(4):
                                    jc = 4 * p + g
                                    h, hg = divmod(g, 2)
                                    nc.tensor.matmul(
                                        halves[h][:, hg * 512: hg * 512 + iw],
                                        k4[32 * g:32 * g + 32,
                                           jc * 128:(jc + 1) * 128],
                                        q4[32 * g:32 * g + 32, i0:i0 + iw],
                                        start=True, stop=True,
                                        tile_position=(32 * g, 0))
                                for g in range(4):
                                    h, hg = divmod(g, 2)
                                    nc.scalar.activation(
                                        E[:, g * 512: g * 512 + iw],
                                        halves[h][:, hg * 512: hg * 512 + iw],
                                        Act.Exp, bias=expbias[:, 0:1])
                                for gp in range(2):
                                    E3 = E[:, gp * 1024:(gp + 1) * 1024].rearrange(
                                        "p (t i) -> p t i", t=2)
                                    apply_pair(2 * p + gp, E3)

                        # ---- softmax normalization + wv + residual ----
                        rsb = small.tile([1, 512], f32, name=f"rsb_{d}_{ib}",
                                         tag="rsb", bufs=2)
                        nc.vector.reciprocal_approx_fast(rsb[:1, :iw], cs1[:1, :iw])
                        msb = small.tile([1, 512], bf, name=f"msb_{d}_{ib}",
                                         tag="msb", bufs=2)
                        nc.vector.tensor_tensor(msb[:1, :iw], rsb[:1, :iw],
                                                mask[:1, i0:i0 + iw], Alu.mult)
                        # bc lives on the "pack" ring (not "cs") so the NEXT
                        # block's csum matmuls are unblocked as soon as the
                        # reciprocal has read cs1, instead of waiting for the
                        # whole normalize chain.
                        bc = psA.tile([128, 1024], f32, name=f"bc_{d}_{ib}",
                                      tag="pack", bufs=2)
                        nc.tensor.matmul(bc[:, :iw], onesk1[:], msb[:1, :iw],
                                         start=True, stop=True)
                        rec = small.tile([128, 512], f32, name=f"rec_{d}_{ib}",
                                         tag="rec", bufs=2)
                        nc.vector.tensor_copy(rec[:, :iw], bc[:, :iw])

                        # Gn = G * (mask/csum), cin-interleaved fp8, then
                        # attn = (gamma*wv) @ Gn via one DR matmul per output
                        # channel half; residual add reads the PSUM directly.
                        tmp_il = small.tile([128, 2, 512], f8,
                                            name=f"tmp_{d}_{ib}", tag="tmp",
                                            bufs=2)
                        for cc in range(2):
                            nc.vector.tensor_tensor(tmp_il[:, cc, :iw],
                                                    ap_ps[cc][:, :iw],
                                                    rec[:, :iw], Alu.mult)
                        nr = iw // 64
                        r0b = i0 // 64
                        for occ in range(2):
                            att = psA.tile([128, 512], f32,
                                           name=f"att_{d}_{ib}_{occ}",
                                           tag="apply", bufs=2)
                            nc.tensor.matmul(
                                att[:, :iw],
                                attw_vTil(d)[:, :, occ * 128:(occ + 1) * 128],
                                tmp_il[:, :, :iw], perf_mode=DR,
                                start=True, stop=True, skip_group_check=True)
                            att3 = att.rearrange("p (r x) -> p r x", x=64)
                            fq3 = fq34[d][occ].rearrange("p (r x) -> p r x", x=64)
                            enh3 = enh[d][occ].rearrange("p (r x) -> p r x", x=WP)
                            nc.vector.scalar_tensor_tensor(
                                enh3[:, r0b:r0b + nr, 2:66],
                                att3[:, :nr, :],
                                scal[:, 4 * d + 2 + occ:4 * d + 3 + occ],
                                fq3[:, r0b:r0b + nr, :],
                                Alu.add, Alu.add)

                # ============ 3x3 conv + BN + ReLU ============
                # 8 waves on the "conv" bank, emitted sp-major with the
                # dir-0 channel taps FIRST inside each wave: the d0 taps only
                # need dir-0's enh, so the scheduler can hoist them into the
                # PE slack of dir-1's ACT-paced attention phase; the d1 taps
                # of wave sp are ready as soon as dir-1's i-blocks reach
                # those rows.
                enh3 = [[enh[d][cc].rearrange("p (r x) -> p r x", x=WP)
                         for cc in range(2)] for d in range(2)]
                for sp in range(4):
                    for oc in range(2):
                        cp = psA.tile([128, 512], f32, name=f"cv_{oc}_{sp}",
                                      tag="conv", bufs=1)
                        first = True
                        for cc4 in range(4):
                            d, cc = divmod(cc4, 2)
                            for ky in range(3):
                                for kx in range(3):
                                    tslot = ((ky * 3 + kx) * 4 + cc4) * 2 + oc
                                    wsl = convw[:, tslot * 128:(tslot + 1) * 128]
                                    rhs = enh3[d][cc][:, sp * 8 + ky: sp * 8 + ky + 8,
                                                      kx + 1: kx + 65]
                                    nc.tensor.matmul(
                                        cp[:], wsl, rhs,
                                        start=first,
                                        stop=(cc4 == 3 and ky == 2 and kx == 2),
                                        skip_group_check=True)
                                    first = False
                        # bias+ReLU on DVE: the ACT engine paces the
                        # attention exps; keep the conv epilogue off it
                        nc.vector.tensor_scalar(
                            y_sb[oc][:, sp * 512:(sp + 1) * 512],
                            cp[:], convb[:, oc:oc + 1], 0.0,
                            Alu.add, Alu.max)
                        nc.sync.dma_start(
                            d_y[oc * 128:(oc + 1) * 128, sp * 512:(sp + 1) * 512],
                            y_sb[oc][:, sp * 512:(sp + 1) * 512])

    nc.compile()
    return nc


def _get_nc():
    if "nc" not in _CACHE:
        _CACHE["nc"] = _build_program()
    return _CACHE["nc"]


def _host_prep(inputs):
    f32 = np.float32
    ii = {k: np.asarray(v, dtype=f32) if np.asarray(v).dtype.kind == "f"
          else np.asarray(v) for k, v in inputs.items()}

    # ---- shared (core-independent) tensors ----
    attwq = np.zeros((2, 128, 256), f32)
    attwkv = np.zeros((2, 128, 768), f32)
    scal = np.zeros((128, 8), f32)
    for d, sfx in enumerate(("d2r", "r2d")):
        wq, bq = ii[f"wq_{sfx}"], ii[f"bq_{sfx}"]
        wk, bk = ii[f"wk_{sfx}"], ii[f"bk_{sfx}"]
        wv, bv = ii[f"wv_{sfx}"], ii[f"bv_{sfx}"]
        g = float(ii[f"gamma_{sfx}"].reshape(-1)[0])
        wq4t = np.tile(wq, (4, 1)).T.astype(f32)   # (256, 128)
        wk4t = np.tile(wk, (4, 1)).T.astype(f32)   # (256, 128)
        wvt = (g * wv).T.astype(f32)               # (256, 256) [cin, cout]
        attwq[d][:, 0:128] = wq4t[0:128]
        attwq[d][:, 128:256] = wq4t[128:256]
        # interleaved fp8: k4T_il[p, t*128+m] = wk4t[t*128+p, m]
        attwkv[d][:, 0:128] = wk4t[0:128]
        attwkv[d][:, 128:256] = wk4t[128:256]
        # wvT_il[p, t*256+c] = wvt[t*128+p, c]
        attwkv[d][:, 256:512] = wvt[0:128]
        attwkv[d][:, 512:768] = wvt[128:256]
        scal[:, 4 * d + 0] = np.tile(bq, 4)
        scal[:, 4 * d + 1] = np.tile(bk, 4)
        scal[:, 4 * d + 2] = g * bv[0:128]
        scal[:, 4 * d + 3] = g * bv[128:256]

    # conv + BN fold
    eps = f32(1e-5)
    inv = (1.0 / np.sqrt(ii["bn_var"] + eps)).astype(f32)
    sc = inv * ii["bn_scale"]
    wf_f = (ii["wf"] * sc[:, None, None, None]).astype(f32)      # (256,512,3,3)
    bf_f = ((ii["bf"] - ii["bn_mean"]) * sc + ii["bn_bias"]).astype(f32)
    convw = np.zeros((128, 72 * 128), f32)
    for ky in range(3):
        for kx in range(3):
            for cc4 in range(4):
                for oc in range(2):
                    t = ((ky * 3 + kx) * 4 + cc4) * 2 + oc
                    blk = wf_f[oc * 128:(oc + 1) * 128,
                               cc4 * 128:(cc4 + 1) * 128, ky, kx]
                    convw[:, t * 128:(t + 1) * 128] = blk.T
    convb = np.stack([bf_f[0:128], bf_f[128:256]], axis=1).astype(f32)  # (128,2)

    shared = {
        "attwq_0": attwq[0].astype(BF16),
        "attwq_1": attwq[1].astype(BF16),
        "attwkv_0": attwkv[0].astype(F8),
        "attwkv_1": attwkv[1].astype(F8),
        "scal": scal,
        "convw": convw.astype(BF16),
        "convb": convb,
    }

    # ---- per-core tensors ----
    f_rgb = ii["f_rgb"].reshape(B, C, H, W)
    f_depth = ii["f_depth"].reshape(B, C, H, W)

    def make34(img, r0):  # img (C,H,W) -> (C, NI) bf16, rows [r0-1, r0+33)
        out = np.zeros((C, 34, W), f32)
        lo = r0 - 1
        s_lo, s_hi = max(lo, 0), min(r0 + 33, H)
        out[:, s_lo - lo: s_hi - lo, :] = img[:, s_lo:s_hi, :]
        return out.reshape(C, NI).astype(BF16)

    def make_il(img):  # (C, H, W) -> (128, 2, N) fp8 channel-interleaved
        flat = img.reshape(C, N)
        return flat.reshape(2, 128, N).transpose(1, 0, 2).astype(F8)

    def make_T(img):   # (C, H, W) -> (128, 16, 2, C) fp8 token-major
        flat = img.reshape(C, N)  # fkvT[p, pr, t, c] = f[c, 256*pr+128*t+p]
        return flat.T.reshape(16, 2, 128, C).transpose(2, 0, 1, 3).astype(F8)

    in_maps = []
    for core in range(8):
        b, half = divmod(core, 2)
        r0 = half * 32
        mask = np.ones((1, NI), f32)
        if half == 0:
            mask[0, 0:64] = 0.0
        else:
            mask[0, NI - 64:NI] = 0.0
        m = dict(shared)
        m["fq34_0"] = make34(f_rgb[b], r0)     # d2r: Q/resid on rgb
        m["fq34_1"] = make34(f_depth[b], r0)   # r2d: Q/resid on depth
        m["fkv_0"] = make_il(f_depth[b])       # d2r K/V
        m["fkv_1"] = make_il(f_rgb[b])         # r2d K/V
        m["fkvT_0"] = make_T(f_depth[b])
        m["fkvT_1"] = make_T(f_rgb[b])
        m["mask"] = mask
        in_maps.append(m)
    return in_maps


def kernel(**inputs):
    global LAST_RESULTS
    from concourse import bass_utils

    nc = _get_nc()
    in_maps = _host_prep(inputs)
    res = bass_utils.run_bass_kernel_spmd(nc, in_maps, core_ids=list(range(8)))
    LAST_RESULTS = res

    y = np.zeros((B, C, H, W), np.float32)
    for core in range(8):
        b, half = divmod(core, 2)
        y[b, :, half * 32:(half + 1) * 32, :] = \
            res.results[core]["y"].astype(np.float32).reshape(C, 32, W)
    return y

